# revision 1
# baseline (speedup 1.0000x reference)
"""LIF spiking-neuron kernel (nn_Neuron_75222057222206) for 8x TRN2 NeuronCores.

Reference semantics (per timestep t, elementwise over [B, N] state):
    u = tau_c * u + x[:, t]        (leaky integration, tau_c = clip(tau,0,1))
    o = (u - 1.0 > 0).float()      (spike)
    u = u * (1.0 - o)              (multiplicative reset)
Output: o stacked over t -> [B, T, N] float32.

Sharding: pure data-parallel over batch. B=32 -> 4 batch rows per core,
zero communication. Per-core state u is [4, 65536] f32 = 1 MB, held in
SBUF as [128 partitions x 2048], (b, n) -> partition b*32 + n//2048,
free n%2048.

The kernel is DMA-byte-bound: per core it must read 32 MB of x and write
the spikes. Compute (3 DVE ops per timestep) is far faster than the DMA
stream. tau is baked in at trace time as an immediate (the kernel is
compiled per call, so this is just compile-time constant specialization -
any tau value works).
"""

import numpy as np

B, T, N = 32, 32, 65536
NCORES = 8
BL = B // NCORES          # batch rows per core (4)
P = 128                   # SBUF partitions
F = (BL * N) // P         # free elements per partition (2048)
QP = N // F               # partitions per batch row (32)
THRESH = 1.0

# test.py may flip this to get an NTFF profile + exec time out of the run.
TRACE = False
LAST_RESULTS = None       # stash of BassKernelResults when TRACE

# Perf variants (validated on HW before becoming defaults):
#  SPIKE_ON_ACT: compute o = Sign(Relu(v-1)) on the ScalarE (ACT) engine
#    instead of a DVE compare. Not needed for speed (DVE has headroom).
#  OUT_DT: on-device spike storage - "f32", "bf16", "u8" (0/1 exact in all
#    three; host casts back to f32 during the unshard), or "pack32"
#    (all 32 timesteps of one neuron bit-packed into one int32; host
#    unpacks - output traffic drops 32x vs f32).
#  CHUNK: timesteps per DMA instruction (1, 2 or 4) - larger transfers can
#    sustain better HBM bandwidth.
SPIKE_ON_ACT = False
OUT_DT = "f32"
CHUNK = 1


def _ensure_import_path():
    import sys
    try:
        import concourse  # noqa: F401
    except ImportError:
        sys.path.insert(0, "/opt/trn_rl_repo")


def build(nc, tau_c: float, reps: int = 1):
    """Emit the per-core LIF kernel into Bass object `nc`.

    reps>1 re-runs the whole T-loop (identical outputs rewritten) - used
    only by the local bench to measure HW exec time differentially."""
    import concourse.mybir as mybir
    import concourse.tile as tile

    f32 = mybir.dt.float32
    i32 = mybir.dt.int32
    Alu = mybir.AluOpType
    Act = mybir.ActivationFunctionType
    pack = OUT_DT == "pack32"
    odt = {"f32": f32, "bf16": mybir.dt.bfloat16, "u8": mybir.dt.uint8,
           "pack32": i32}[OUT_DT]
    C = CHUNK
    assert T % C == 0
    nG = T // C

    x_d = nc.dram_tensor("x", [BL, T, N], f32, kind="ExternalInput")
    if pack:
        o_d = nc.dram_tensor("o", [BL, N], i32, kind="ExternalOutput")
        o_r = o_d.ap().rearrange("b (q f) -> b q f", f=F)
    else:
        o_d = nc.dram_tensor("o", [BL, T, N], odt, kind="ExternalOutput")
        # [BL, T, N] -> [nG, BL, QP, C, F]: group g is the DRAM side of a
        # [128, C*F] SBUF tile (partition dim = (b, q), free = (c, f)).
        o_r = o_d.ap().rearrange("b (g c) (q f) -> g b q c f", c=C, f=F)
    x_r = x_d.ap().rearrange("b (g c) (q f) -> g b q c f", c=C, f=F)

    xbufs = {1: 8, 2: 5, 4: 3}[C]
    obufs = {1: 8, 2: 5, 4: 3}[C]
    with tile.TileContext(nc) as tc:
        with (
            tc.tile_pool(name="xp", bufs=xbufs) as xp,
            tc.tile_pool(name="op", bufs=obufs) as op,
            tc.tile_pool(name="vp", bufs=3) as vp,
            tc.tile_pool(name="up", bufs=2) as up,
            tc.tile_pool(name="rp", bufs=3) as rp,
            tc.tile_pool(name="o32", bufs=2) as o32p,
            tc.tile_pool(name="accp", bufs=2) as accp,
            tc.tile_pool(name="cp", bufs=1) as cp,
        ):
            negth = None
            if SPIKE_ON_ACT:
                negth = cp.tile([P, 1], f32)
                nc.vector.memset(negth[:], -THRESH)
            tsh = None
            if pack:
                # per-partition int32 shift amounts (walrus requires the
                # bitvec scalar operand to be integer-typed, so no imm)
                tsh = cp.tile([P, T], i32, tag="tsh")
                for tt in range(T):
                    nc.vector.memset(tsh[:, tt:tt + 1], tt)
            for rep in range(reps):
                u = None
                acc = None
                for g in range(nG):
                    xt = xp.tile([P, C * F], f32)
                    nc.sync.dma_start(xt[:], x_r[g])
                    ot = None
                    if not pack:
                        ot = op.tile([P, C * F], odt)
                    for s in range(C):
                        t = g * C + s
                        xs = xt[:, s * F:(s + 1) * F]
                        if t == 0:
                            v = xs  # u0 == 0 so v = x[0]
                        else:
                            vt = vp.tile([P, F], f32)
                            nc.vector.scalar_tensor_tensor(
                                vt[:], u, tau_c, xs, Alu.mult, Alu.add
                            )
                            v = vt[:]
                        if pack:
                            if t == 0:
                                acc = accp.tile([P, F], i32)
                                nc.vector.tensor_scalar(
                                    acc[:], v, THRESH, None, Alu.is_gt
                                )
                            else:
                                ob = o32p.tile([P, F], i32)
                                nc.vector.tensor_scalar(
                                    ob[:], v, THRESH, None, Alu.is_gt
                                )
                                acc2 = accp.tile([P, F], i32)
                                nc.vector.scalar_tensor_tensor(
                                    acc2[:], ob[:], tsh[:, t:t + 1], acc[:],
                                    Alu.arith_shift_left, Alu.bitwise_or,
                                )
                                acc = acc2
                        else:
                            ov = ot[:, s * F:(s + 1) * F]
                            if SPIKE_ON_ACT:
                                rt = rp.tile([P, F], f32)
                                nc.scalar.activation(rt[:], v, Act.Relu,
                                                     bias=negth[:], scale=1.0)
                                nc.scalar.activation(ov, rt[:], Act.Sign)
                            else:
                                nc.vector.tensor_scalar(ov, v, THRESH,
                                                        None, Alu.is_gt)
                        if t != T - 1:
                            u2 = up.tile([P, F], f32)
                            nc.vector.scalar_tensor_tensor(
                                u2[:], v, THRESH, v, Alu.is_le, Alu.mult
                            )
                            u = u2[:]
                    if not pack:
                        # Stores go on ACT's HWDGE ring: a separate FIFO from
                        # the loads so a store blocked on compute can't
                        # head-block prefetch loads.
                        nc.scalar.dma_start(o_r[g], ot[:])
                if pack:
                    nc.scalar.dma_start(o_r, acc[:])
    return x_d, o_d


def make_nc(tau_c: float, reps: int = 1):
    _ensure_import_path()
    from concourse import bacc

    nc = bacc.Bacc("TRN2", target_bir_lowering=False, debug=False)
    build(nc, tau_c, reps=reps)
    nc.compile()
    return nc


def _unpack_bits(acc):
    # acc: [B, N] int32 -> [B, T, N] f32, bit t = spike at timestep t
    shifts = np.arange(T, dtype=np.int32)[None, :, None]
    return ((acc[:, None, :] >> shifts) & 1).astype(np.float32)


def kernel(x, tau):
    global LAST_RESULTS
    _ensure_import_path()
    from concourse.bass_utils import run_bass_kernel_spmd

    x = np.ascontiguousarray(np.asarray(x, dtype=np.float32))
    tau_c = float(np.clip(np.asarray(tau, dtype=np.float32), 0.0, 1.0).ravel()[0])
    assert x.shape == (B, T, N), x.shape

    nc = make_nc(tau_c)
    in_maps = [{"x": x[c * BL : (c + 1) * BL]} for c in range(NCORES)]
    res = run_bass_kernel_spmd(nc, in_maps, list(range(NCORES)), trace=TRACE)
    LAST_RESULTS = res
    per_core = [res.results[c]["o"] for c in range(NCORES)]
    if OUT_DT == "pack32":
        out = np.concatenate(
            [_unpack_bits(p.view(np.int32) if p.dtype != np.int32 else p)
             for p in per_core], axis=0)
    else:
        out = np.concatenate(per_core, axis=0)
        if out.dtype != np.float32:
            out = out.astype(np.float32)
    return out

